# revision 10
# baseline (speedup 1.0000x reference)
"""BatchHardTripletLoss on 8 Trainium2 NeuronCores.

Strategy (data parallel over rows; all reductions in squared-distance space;
sqrt is monotone so squared-space hardest-pos/neg selection is exact):

  Host: sort rows by label. Core c owns sorted rows [1024c, 1024c+1024).
  Columns (all 8192 candidates) are rotated per core so its own rows sit at
  fixed local columns [W/2, W/2+1024) -> every row-tile's same-class columns
  fall in a fixed local window => one SPMD program for all 8 cores.

  Two device pipelines per core, split by column region:
   1) Row path (local cols [0,1536) u [3584,8192), includes the class band):
      TensorE: psum[i,j] = sq_j - 2 x_i.x_j  (f32r matmul with -2x rows as
      stationary + rank-1 ones @ sq/128 matmul accumulating sq_j);
      VectorE: min-reduce per 1536-col group; the band window gets +/-1e30
      label masks (tensor_tensor add) for hardest-neg / hardest-pos.
   2) Transposed path (local cols [1536,3584), guaranteed band-free):
      TensorE: psum[j,i] = x_j.x_i for 16 j-tiles x all 1024 own rows;
      ScalarE: tbuf = 2*psum - sq_j (per-partition bias, Identity activation);
      GpSimd:  partition_all_reduce(max) over the 128 j's -> per-jt row
      maxima, shipped to host which negates (min = -max(-t)) and combines.

  Host epilogue: + sq_i, clamp, sqrt (eps rule), validity from label counts
  (self-inclusion in hardest-pos is harmless: singleton classes are invalid
  by count), margin + masked mean in fp32.
"""

import numpy as np

N = 8192
D = 128
MARGIN = 0.3
NCORES = 8
ROWS_PER_CORE = N // NCORES          # 1024
RT_PER_CORE = ROWS_PER_CORE // 128   # 8 row-tiles
RW = 1536                            # row-path psum group width (3 banks)
TR0 = 1536                           # transposed region start (local cols)
TRN = 16                             # transposed j-tiles (128 each)
ROW_GROUPS = [(0, 1536), (3584, 5120), (5120, 6656), (6656, 8192)]
MMN = 512
BIG = 1.0e30

_PROGRAM_CACHE = {}


def _build_program(W):
    import concourse.mybir as mybir
    import concourse.bass_isa as bass_isa
    from concourse import bacc
    from concourse.tile import TileContext

    F32 = mybir.dt.float32
    F32R = mybir.dt.float32r

    nc = bacc.Bacc("TRN2", target_bir_lowering=False, debug=False,
                   num_devices=NCORES)

    featsT_d = nc.dram_tensor("featsT", [D, N], F32R, kind="ExternalInput")
    sqb_d = nc.dram_tensor("sqb", [D, 4 * RW], F32R, kind="ExternalInput")
    ones_d = nc.dram_tensor("ones", [D, 128], F32R, kind="ExternalInput")
    rows2_d = nc.dram_tensor("rows2", [D, ROWS_PER_CORE], F32R,
                             kind="ExternalInput")
    negsq_d = nc.dram_tensor("negsq", [D, TRN], F32, kind="ExternalInput")
    negmask_d = nc.dram_tensor("negmask", [D, RT_PER_CORE * W], F32,
                               kind="ExternalInput")
    posmask_d = nc.dram_tensor("posmask", [D, RT_PER_CORE * W], F32,
                               kind="ExternalInput")
    neg_out_d = nc.dram_tensor("neg_out", [D, RT_PER_CORE], F32,
                               kind="ExternalOutput")
    pos_out_d = nc.dram_tensor("pos_out", [D, RT_PER_CORE], F32,
                               kind="ExternalOutput")
    gneg_out_d = nc.dram_tensor("gneg_out", [TRN, ROWS_PER_CORE], F32,
                                kind="ExternalOutput")

    with TileContext(nc) as tc:
        with (
            tc.tile_pool(name="big", bufs=1) as big,
            tc.tile_pool(name="rps", bufs=2, space="PSUM") as rps_pool,
            tc.tile_pool(name="tps", bufs=2, space="PSUM") as tps_pool,
            tc.tile_pool(name="scr", bufs=4) as scr,
            tc.tile_pool(name="tb", bufs=2) as tb_pool,
            tc.tile_pool(name="small", bufs=1) as small,
        ):
            featsT = big.tile([D, N], F32R, tag="featsT")
            sqb = big.tile([D, 4 * RW], F32R, tag="sqb")
            ones = small.tile([D, 128], F32R, tag="ones")
            rows2 = big.tile([D, ROWS_PER_CORE], F32R, tag="rows2")
            negsq = small.tile([D, TRN], F32, tag="negsq")
            negmask = big.tile([D, RT_PER_CORE * W], F32, tag="negmask")
            posmask = big.tile([D, RT_PER_CORE * W], F32, tag="posmask")
            # critical-path first; spread issue across the 3 DMA-capable queues
            nc.sync.dma_start(ones[:, :], ones_d[:, :])
            nc.gpsimd.dma_start(rows2[:, :], rows2_d[:, :])
            nc.scalar.dma_start(negsq[:, :], negsq_d[:, :])
            for ch in range(8):
                sl = slice(ch * 1024, (ch + 1) * 1024)
                eng = nc.sync if ch % 2 == 0 else nc.gpsimd
                eng.dma_start(featsT[:, sl], featsT_d[:, sl])
            for ch in range(4):
                sl = slice(ch * RW, (ch + 1) * RW)
                nc.scalar.dma_start(sqb[:, sl], sqb_d[:, sl])
            nc.sync.dma_start(negmask[:, :], negmask_d[:, :])
            nc.sync.dma_start(posmask[:, :], posmask_d[:, :])

            neg_sb = small.tile([D, RT_PER_CORE], F32, tag="neg_sb")
            pos_sb = small.tile([D, RT_PER_CORE], F32, tag="pos_sb")

            def emit_transposed(jt):
                tbuf = tb_pool.tile([D, ROWS_PER_CORE], F32, tag="tbuf",
                                    name=f"tbuf{jt}")
                lhsT = featsT[:, TR0 + 128 * jt:TR0 + 128 * (jt + 1)]
                for h in range(2):
                    ps_t = tps_pool.tile([D, MMN], F32, tag="ps_t",
                                         name=f"ps_t{jt}_{h}")
                    nc.tensor.matmul(
                        ps_t[:, :], lhsT,
                        featsT[:, W // 2 + h * MMN:W // 2 + (h + 1) * MMN],
                        start=True, stop=True)
                    nc.scalar.activation(
                        tbuf[:, h * MMN:(h + 1) * MMN], ps_t[:, :],
                        mybir.ActivationFunctionType.Identity,
                        bias=negsq[:, jt:jt + 1], scale=2.0)
                gout = tb_pool.tile([D, ROWS_PER_CORE], F32, tag="gout",
                                    name=f"gout{jt}")
                nc.gpsimd.partition_all_reduce(
                    gout[:, :], tbuf[:, :], 128, bass_isa.ReduceOp.max)
                nc.sync.dma_start(gneg_out_d[jt:jt + 1, :], gout[0:1, :])

            for lt in range(RT_PER_CORE):
                lhsT = rows2[:, 128 * lt:128 * (lt + 1)]
                partials = scr.tile([D, 4], F32, tag="partials",
                                    name=f"partials{lt}")
                w0 = 128 * lt + 64
                for g, (c0g, c1g) in enumerate(ROW_GROUPS):
                    ps = rps_pool.tile([D, RW], F32, tag="ps",
                                       name=f"ps{lt}_{g}")
                    for k in range(RW // MMN):
                        c0 = c0g + k * MMN
                        sq0 = g * RW + k * MMN
                        nc.tensor.matmul(
                            ps[:, k * MMN:(k + 1) * MMN], lhsT,
                            featsT[:, c0:c0 + MMN], start=True, stop=False)
                        nc.tensor.matmul(
                            ps[:, k * MMN:(k + 1) * MMN], ones,
                            sqb[:, sq0:sq0 + MMN], start=False, stop=True)
                    if g == 0:
                        scrP = scr.tile([D, W], F32, tag="scrP",
                                        name=f"scrP{lt}")
                        nc.vector.tensor_tensor(
                            out=scrP[:, :], in0=ps[:, w0:w0 + W],
                            in1=posmask[:, lt * W:(lt + 1) * W],
                            op=mybir.AluOpType.add)
                        nc.vector.tensor_reduce(
                            pos_sb[:, lt:lt + 1], scrP[:, :],
                            axis=mybir.AxisListType.X, op=mybir.AluOpType.max)
                        nc.vector.tensor_tensor(
                            out=ps[:, w0:w0 + W], in0=ps[:, w0:w0 + W],
                            in1=negmask[:, lt * W:(lt + 1) * W],
                            op=mybir.AluOpType.add)
                    nc.vector.tensor_reduce(
                        partials[:, g:g + 1], ps[:, :],
                        axis=mybir.AxisListType.X, op=mybir.AluOpType.min)
                nc.vector.tensor_reduce(
                    neg_sb[:, lt:lt + 1], partials[:, 0:4],
                    axis=mybir.AxisListType.X, op=mybir.AluOpType.min)
                emit_transposed(2 * lt)
                emit_transposed(2 * lt + 1)

            nc.sync.dma_start(neg_out_d[:, :], neg_sb[:, :])
            nc.sync.dma_start(pos_out_d[:, :], pos_sb[:, :])

    nc.compile()
    return nc


def kernel(feats, labels):
    from concourse.bass_utils import run_bass_kernel_spmd

    feats = np.asarray(feats, dtype=np.float32)
    labels_np = np.asarray(labels).astype(np.int64)

    order = np.argsort(labels_np, kind="stable")
    feats_s = feats[order]
    labels_s = labels_np[order]

    counts = np.bincount(labels_s, minlength=max(int(labels_s.max()) + 1, 1))
    mc = int(counts.max())
    if mc <= 65:
        W = 256
    elif mc <= 129:
        W = 384
    elif mc <= 193:
        W = 512
    else:
        raise ValueError(f"class of size {mc} exceeds supported band window")

    if W not in _PROGRAM_CACHE:
        _PROGRAM_CACHE[W] = _build_program(W)
    nc = _PROGRAM_CACHE[W]

    sq = np.einsum("nd,nd->n", feats_s.astype(np.float64),
                   feats_s.astype(np.float64)).astype(np.float32)
    ones_np = np.ones((D, 128), dtype=np.float32)

    in_maps = []
    for c in range(NCORES):
        rot = (ROWS_PER_CORE * c - W // 2) % N
        loc = (rot + np.arange(N)) % N          # local col -> global sorted row
        featsT_c = np.ascontiguousarray(feats_s[loc].T)
        rows2_c = np.ascontiguousarray(
            (-2.0 * feats_s[ROWS_PER_CORE * c:ROWS_PER_CORE * (c + 1)]).T)
        sq_loc = sq[loc]
        rp_cols = np.concatenate([np.arange(a, b) for a, b in ROW_GROUPS])
        sqb_c = np.ascontiguousarray(
            np.broadcast_to((sq_loc[rp_cols] / 128.0)[None, :], (D, 4 * RW)))
        negsq_c = np.ascontiguousarray(
            -sq_loc[TR0:TR0 + TRN * 128].reshape(TRN, 128).T)
        negmask_c = np.zeros((D, RT_PER_CORE * W), dtype=np.float32)
        posmask_c = np.zeros((D, RT_PER_CORE * W), dtype=np.float32)
        for lt in range(RT_PER_CORE):
            rows_lab = labels_s[ROWS_PER_CORE * c + 128 * lt:
                                ROWS_PER_CORE * c + 128 * (lt + 1)]
            w0 = 128 * lt + 64
            win_lab = labels_s[loc[w0:w0 + W]]
            same = rows_lab[:, None] == win_lab[None, :]
            negmask_c[:, lt * W:(lt + 1) * W] = np.where(same, BIG, 0.0)
            posmask_c[:, lt * W:(lt + 1) * W] = np.where(same, 0.0, -BIG)
        in_maps.append({
            "featsT": featsT_c,
            "sqb": sqb_c,
            "ones": ones_np,
            "rows2": rows2_c,
            "negsq": negsq_c,
            "negmask": negmask_c,
            "posmask": posmask_c,
        })

    res = run_bass_kernel_spmd(nc, in_maps, core_ids=list(range(NCORES)))

    neg_raw = np.empty(N, dtype=np.float32)
    pos_raw = np.empty(N, dtype=np.float32)
    for c in range(NCORES):
        base = ROWS_PER_CORE * c
        nr = res.results[c]["neg_out"].T.reshape(ROWS_PER_CORE)
        tr = -res.results[c]["gneg_out"].max(axis=0)   # min over transposed j
        neg_raw[base:base + ROWS_PER_CORE] = np.minimum(nr, tr)
        pos_raw[base:base + ROWS_PER_CORE] = \
            res.results[c]["pos_out"].T.reshape(ROWS_PER_CORE)

    hn_sq = np.maximum(neg_raw + sq, 0.0).astype(np.float32)
    hp_sq = np.maximum(pos_raw + sq, 0.0).astype(np.float32)
    eps = np.float32(1e-12)
    hn = np.where(hn_sq > eps, np.sqrt(hn_sq), np.float32(0.0))
    hp = np.where(hp_sq > eps, np.sqrt(hp_sq), np.float32(0.0))

    cnt_per_row = counts[labels_s]
    valid = (cnt_per_row >= 2) & (cnt_per_row < N)
    diff = np.where(valid, hp - hn, np.float32(0.0))
    per_row = np.maximum(diff + np.float32(MARGIN), np.float32(0.0))
    per_row = np.where(valid, per_row, np.float32(0.0)).astype(np.float32)
    cnt = np.float32(valid.sum())
    if cnt > 0:
        loss = np.float32(per_row.sum(dtype=np.float32) / max(cnt, np.float32(1.0)))
    else:
        loss = np.float32(0.0)
    return np.float32(loss)


# revision 12
# speedup vs baseline: 1.1453x; 1.1453x over previous
"""BatchHardTripletLoss on 8 Trainium2 NeuronCores.

Strategy (data parallel over rows; all reductions in squared-distance space;
sqrt is monotone so squared-space hardest-pos/neg selection is exact):

  Host: sort rows by label. Core c owns sorted rows [1024c, 1024c+1024).
  Columns (all 8192 candidates) are rotated per core so its own rows sit at
  fixed local columns [W/2, W/2+1024) -> every row-tile's same-class columns
  fall in a fixed local window => one SPMD program for all 8 cores.

  Two device pipelines per core, split by column region:
   1) Row path (local cols [0,1536) u [3584,8192), includes the class band):
      TensorE: psum[i,j] = sq_j - 2 x_i.x_j  (f32r matmul with -2x rows as
      stationary + rank-1 ones @ sq/128 matmul accumulating sq_j);
      VectorE: min-reduce per 1536-col group; the band window gets +/-1e30
      label masks (tensor_tensor add) for hardest-neg / hardest-pos.
   2) Transposed path (local cols [1536,3584), guaranteed band-free):
      TensorE: psum[j,i] = x_j.x_i for 16 j-tiles x all 1024 own rows;
      ScalarE: tbuf = 2*psum - sq_j (per-partition bias, Identity activation);
      GpSimd:  partition_all_reduce(max) over the 128 j's -> per-jt row
      maxima, shipped to host which negates (min = -max(-t)) and combines.

  Host epilogue: + sq_i, clamp, sqrt (eps rule), validity from label counts
  (self-inclusion in hardest-pos is harmless: singleton classes are invalid
  by count), margin + masked mean in fp32.
"""

import numpy as np

N = 8192
D = 128
MARGIN = 0.3
NCORES = 8
ROWS_PER_CORE = N // NCORES          # 1024
RT_PER_CORE = ROWS_PER_CORE // 128   # 8 row-tiles
RW = 1536                            # row-path psum group width (3 banks)
TR0 = 1536                           # transposed region start (local cols)
TRN = 16                             # transposed j-tiles (128 each)
ROW_GROUPS = [(0, 1536), (3584, 5120), (5120, 6656), (6656, 8192)]
MMN = 512
BIG = 1.0e30

_PROGRAM_CACHE = {}


def _build_program(W):
    import concourse.mybir as mybir
    import concourse.bass_isa as bass_isa
    from concourse import bacc
    from concourse.tile import TileContext

    F32 = mybir.dt.float32
    F32R = mybir.dt.float32r

    nc = bacc.Bacc("TRN2", target_bir_lowering=False, debug=False,
                   num_devices=NCORES)

    featsT_d = nc.dram_tensor("featsT", [D, N], F32R, kind="ExternalInput")
    sqb_d = nc.dram_tensor("sqb", [D, 4 * RW], F32R, kind="ExternalInput")
    ones_d = nc.dram_tensor("ones", [D, 128], F32R, kind="ExternalInput")
    rows2_d = nc.dram_tensor("rows2", [D, ROWS_PER_CORE], F32R,
                             kind="ExternalInput")
    negsq_d = nc.dram_tensor("negsq", [D, TRN], F32, kind="ExternalInput")
    negmask_d = nc.dram_tensor("negmask", [D, RT_PER_CORE * W], F32,
                               kind="ExternalInput")
    posmask_d = nc.dram_tensor("posmask", [D, RT_PER_CORE * W], F32,
                               kind="ExternalInput")
    neg_out_d = nc.dram_tensor("neg_out", [D, RT_PER_CORE], F32,
                               kind="ExternalOutput")
    pos_out_d = nc.dram_tensor("pos_out", [D, RT_PER_CORE], F32,
                               kind="ExternalOutput")
    gneg_out_d = nc.dram_tensor("gneg_out", [TRN // 4, 4 * ROWS_PER_CORE], F32,
                                kind="ExternalOutput")

    with TileContext(nc) as tc:
        with (
            tc.tile_pool(name="big", bufs=1) as big,
            tc.tile_pool(name="rps", bufs=2, space="PSUM") as rps_pool,
            tc.tile_pool(name="tps", bufs=2, space="PSUM") as tps_pool,
            tc.tile_pool(name="scr", bufs=4) as scr,
            tc.tile_pool(name="tb", bufs=2) as tb_pool,
            tc.tile_pool(name="small", bufs=1) as small,
        ):
            featsT = big.tile([D, N], F32R, tag="featsT")
            sqb = big.tile([D, 4 * RW], F32R, tag="sqb")
            ones = small.tile([D, 128], F32R, tag="ones")
            rows2 = big.tile([D, ROWS_PER_CORE], F32R, tag="rows2")
            negsq = small.tile([D, TRN], F32, tag="negsq")
            negmask = big.tile([D, RT_PER_CORE * W], F32, tag="negmask")
            posmask = big.tile([D, RT_PER_CORE * W], F32, tag="posmask")
            # critical-path first; spread issue across the 3 DMA-capable queues
            nc.sync.dma_start(ones[:, :], ones_d[:, :])
            nc.gpsimd.dma_start(rows2[:, :], rows2_d[:, :])
            nc.scalar.dma_start(negsq[:, :], negsq_d[:, :])
            for ch in range(8):
                sl = slice(ch * 1024, (ch + 1) * 1024)
                eng = nc.sync if ch % 2 == 0 else nc.gpsimd
                eng.dma_start(featsT[:, sl], featsT_d[:, sl])
            for ch in range(4):
                sl = slice(ch * RW, (ch + 1) * RW)
                nc.scalar.dma_start(sqb[:, sl], sqb_d[:, sl])
            nc.sync.dma_start(negmask[:, :], negmask_d[:, :])
            nc.sync.dma_start(posmask[:, :], posmask_d[:, :])

            neg_sb = small.tile([D, RT_PER_CORE], F32, tag="neg_sb")
            pos_sb = small.tile([D, RT_PER_CORE], F32, tag="pos_sb")

            def emit_transposed(q):
                # 4 j-tiles -> one concatenated tbuf -> one partition_all_reduce
                tbuf = tb_pool.tile([D, 4 * ROWS_PER_CORE], F32, tag="tbuf",
                                    name=f"tbuf{q}")
                for u in range(4):
                    jt = 4 * q + u
                    lhsT = featsT[:, TR0 + 128 * jt:TR0 + 128 * (jt + 1)]
                    for h in range(2):
                        ps_t = tps_pool.tile([D, MMN], F32, tag="ps_t",
                                             name=f"ps_t{jt}_{h}")
                        nc.tensor.matmul(
                            ps_t[:, :], lhsT,
                            featsT[:, W // 2 + h * MMN:W // 2 + (h + 1) * MMN],
                            start=True, stop=True)
                        nc.scalar.activation(
                            tbuf[:, u * ROWS_PER_CORE + h * MMN:
                                 u * ROWS_PER_CORE + (h + 1) * MMN],
                            ps_t[:, :],
                            mybir.ActivationFunctionType.Identity,
                            bias=negsq[:, jt:jt + 1], scale=2.0)
                gout = tb_pool.tile([D, 4 * ROWS_PER_CORE], F32, tag="gout",
                                    name=f"gout{q}")
                nc.gpsimd.partition_all_reduce(
                    gout[:, :], tbuf[:, :], 128, bass_isa.ReduceOp.max)
                nc.sync.dma_start(gneg_out_d[q:q + 1, :], gout[0:1, :])

            for lt in range(RT_PER_CORE):
                lhsT = rows2[:, 128 * lt:128 * (lt + 1)]
                partials = scr.tile([D, 4], F32, tag="partials",
                                    name=f"partials{lt}")
                w0 = 128 * lt + 64
                for g, (c0g, c1g) in enumerate(ROW_GROUPS):
                    ps = rps_pool.tile([D, RW], F32, tag="ps",
                                       name=f"ps{lt}_{g}")
                    for k in range(RW // MMN):
                        c0 = c0g + k * MMN
                        sq0 = g * RW + k * MMN
                        nc.tensor.matmul(
                            ps[:, k * MMN:(k + 1) * MMN], lhsT,
                            featsT[:, c0:c0 + MMN], start=True, stop=False)
                        nc.tensor.matmul(
                            ps[:, k * MMN:(k + 1) * MMN], ones,
                            sqb[:, sq0:sq0 + MMN], start=False, stop=True)
                    if g == 0:
                        scrP = scr.tile([D, W], F32, tag="scrP",
                                        name=f"scrP{lt}")
                        nc.vector.tensor_tensor(
                            out=scrP[:, :], in0=ps[:, w0:w0 + W],
                            in1=posmask[:, lt * W:(lt + 1) * W],
                            op=mybir.AluOpType.add)
                        nc.vector.tensor_reduce(
                            pos_sb[:, lt:lt + 1], scrP[:, :],
                            axis=mybir.AxisListType.X, op=mybir.AluOpType.max)
                        nc.vector.tensor_tensor(
                            out=ps[:, w0:w0 + W], in0=ps[:, w0:w0 + W],
                            in1=negmask[:, lt * W:(lt + 1) * W],
                            op=mybir.AluOpType.add)
                    nc.vector.tensor_reduce(
                        partials[:, g:g + 1], ps[:, :],
                        axis=mybir.AxisListType.X, op=mybir.AluOpType.min)
                nc.vector.tensor_reduce(
                    neg_sb[:, lt:lt + 1], partials[:, 0:4],
                    axis=mybir.AxisListType.X, op=mybir.AluOpType.min)
                if lt % 2 == 1:
                    emit_transposed(lt // 2)

            nc.sync.dma_start(neg_out_d[:, :], neg_sb[:, :])
            nc.sync.dma_start(pos_out_d[:, :], pos_sb[:, :])

    nc.compile()
    return nc


def kernel(feats, labels):
    from concourse.bass_utils import run_bass_kernel_spmd

    feats = np.asarray(feats, dtype=np.float32)
    labels_np = np.asarray(labels).astype(np.int64)

    order = np.argsort(labels_np, kind="stable")
    feats_s = feats[order]
    labels_s = labels_np[order]

    counts = np.bincount(labels_s, minlength=max(int(labels_s.max()) + 1, 1))
    mc = int(counts.max())
    if mc <= 65:
        W = 256
    elif mc <= 129:
        W = 384
    elif mc <= 193:
        W = 512
    else:
        raise ValueError(f"class of size {mc} exceeds supported band window")

    if W not in _PROGRAM_CACHE:
        _PROGRAM_CACHE[W] = _build_program(W)
    nc = _PROGRAM_CACHE[W]

    sq = np.einsum("nd,nd->n", feats_s.astype(np.float64),
                   feats_s.astype(np.float64)).astype(np.float32)
    ones_np = np.ones((D, 128), dtype=np.float32)

    in_maps = []
    for c in range(NCORES):
        rot = (ROWS_PER_CORE * c - W // 2) % N
        loc = (rot + np.arange(N)) % N          # local col -> global sorted row
        featsT_c = np.ascontiguousarray(feats_s[loc].T)
        rows2_c = np.ascontiguousarray(
            (-2.0 * feats_s[ROWS_PER_CORE * c:ROWS_PER_CORE * (c + 1)]).T)
        sq_loc = sq[loc]
        rp_cols = np.concatenate([np.arange(a, b) for a, b in ROW_GROUPS])
        sqb_c = np.ascontiguousarray(
            np.broadcast_to((sq_loc[rp_cols] / 128.0)[None, :], (D, 4 * RW)))
        negsq_c = np.ascontiguousarray(
            -sq_loc[TR0:TR0 + TRN * 128].reshape(TRN, 128).T)
        negmask_c = np.zeros((D, RT_PER_CORE * W), dtype=np.float32)
        posmask_c = np.zeros((D, RT_PER_CORE * W), dtype=np.float32)
        for lt in range(RT_PER_CORE):
            rows_lab = labels_s[ROWS_PER_CORE * c + 128 * lt:
                                ROWS_PER_CORE * c + 128 * (lt + 1)]
            w0 = 128 * lt + 64
            win_lab = labels_s[loc[w0:w0 + W]]
            same = rows_lab[:, None] == win_lab[None, :]
            negmask_c[:, lt * W:(lt + 1) * W] = np.where(same, BIG, 0.0)
            posmask_c[:, lt * W:(lt + 1) * W] = np.where(same, 0.0, -BIG)
        in_maps.append({
            "featsT": featsT_c,
            "sqb": sqb_c,
            "ones": ones_np,
            "rows2": rows2_c,
            "negsq": negsq_c,
            "negmask": negmask_c,
            "posmask": posmask_c,
        })

    res = run_bass_kernel_spmd(nc, in_maps, core_ids=list(range(NCORES)))

    neg_raw = np.empty(N, dtype=np.float32)
    pos_raw = np.empty(N, dtype=np.float32)
    for c in range(NCORES):
        base = ROWS_PER_CORE * c
        nr = res.results[c]["neg_out"].T.reshape(ROWS_PER_CORE)
        tr = -res.results[c]["gneg_out"].reshape(TRN, ROWS_PER_CORE).max(axis=0)
        neg_raw[base:base + ROWS_PER_CORE] = np.minimum(nr, tr)
        pos_raw[base:base + ROWS_PER_CORE] = \
            res.results[c]["pos_out"].T.reshape(ROWS_PER_CORE)

    hn_sq = np.maximum(neg_raw + sq, 0.0).astype(np.float32)
    hp_sq = np.maximum(pos_raw + sq, 0.0).astype(np.float32)
    eps = np.float32(1e-12)
    hn = np.where(hn_sq > eps, np.sqrt(hn_sq), np.float32(0.0))
    hp = np.where(hp_sq > eps, np.sqrt(hp_sq), np.float32(0.0))

    cnt_per_row = counts[labels_s]
    valid = (cnt_per_row >= 2) & (cnt_per_row < N)
    diff = np.where(valid, hp - hn, np.float32(0.0))
    per_row = np.maximum(diff + np.float32(MARGIN), np.float32(0.0))
    per_row = np.where(valid, per_row, np.float32(0.0)).astype(np.float32)
    cnt = np.float32(valid.sum())
    if cnt > 0:
        loss = np.float32(per_row.sum(dtype=np.float32) / max(cnt, np.float32(1.0)))
    else:
        loss = np.float32(0.0)
    return np.float32(loss)


# revision 13
# speedup vs baseline: 1.1625x; 1.0150x over previous
"""BatchHardTripletLoss on 8 Trainium2 NeuronCores.

Strategy (data parallel over rows; all reductions in squared-distance space;
sqrt is monotone so squared-space hardest-pos/neg selection is exact):

  Host: sort rows by label. Core c owns sorted rows [1024c, 1024c+1024).
  Columns (all 8192 candidates) are rotated per core so its own rows sit at
  fixed local columns [W/2, W/2+1024) -> every row-tile's same-class columns
  fall in a fixed local window => one SPMD program for all 8 cores.

  Two device pipelines per core, split by column region:
   1) Row path (local cols [0,1536) u [3584,8192), includes the class band):
      TensorE: psum[i,j] = sq_j - 2 x_i.x_j  (f32r matmul with -2x rows as
      stationary + rank-1 ones @ sq/128 matmul accumulating sq_j);
      VectorE: min-reduce per 1536-col group; the band window gets +/-1e30
      label masks (tensor_tensor add) for hardest-neg / hardest-pos.
   2) Transposed path (local cols [1536,3584), guaranteed band-free):
      TensorE: psum[j,i] = x_j.x_i for 16 j-tiles x all 1024 own rows;
      ScalarE: tbuf = 2*psum - sq_j (per-partition bias, Identity activation);
      GpSimd:  partition_all_reduce(max) over the 128 j's -> per-jt row
      maxima, shipped to host which negates (min = -max(-t)) and combines.

  Host epilogue: + sq_i, clamp, sqrt (eps rule), validity from label counts
  (self-inclusion in hardest-pos is harmless: singleton classes are invalid
  by count), margin + masked mean in fp32.
"""

import numpy as np

N = 8192
D = 128
MARGIN = 0.3
NCORES = 8
ROWS_PER_CORE = N // NCORES          # 1024
RT_PER_CORE = ROWS_PER_CORE // 128   # 8 row-tiles
RW = 1536                            # row-path psum group width (3 banks)
TR0 = 1536                           # transposed region start (local cols)
TRN = 16                             # transposed j-tiles (128 each)
ROW_GROUPS = [(0, 1536), (3584, 5120), (5120, 6656), (6656, 8192)]
MMN = 512
BIG = 1.0e30

_PROGRAM_CACHE = {}


def _build_program(W):
    import concourse.mybir as mybir
    import concourse.bass_isa as bass_isa
    from concourse import bacc
    from concourse.tile import TileContext

    F32 = mybir.dt.float32
    F32R = mybir.dt.float32r

    nc = bacc.Bacc("TRN2", target_bir_lowering=False, debug=False,
                   num_devices=NCORES)

    featsT_d = nc.dram_tensor("featsT", [D, N], F32R, kind="ExternalInput")
    sqb_d = nc.dram_tensor("sqb", [D, 4 * RW], F32R, kind="ExternalInput")
    ones_d = nc.dram_tensor("ones", [D, 128], F32R, kind="ExternalInput")
    rows2_d = nc.dram_tensor("rows2", [D, ROWS_PER_CORE], F32R,
                             kind="ExternalInput")
    negsq_d = nc.dram_tensor("negsq", [D, TRN], F32, kind="ExternalInput")
    negmask_d = nc.dram_tensor("negmask", [D, RT_PER_CORE * W], F32,
                               kind="ExternalInput")
    posmask_d = nc.dram_tensor("posmask", [D, RT_PER_CORE * W], F32,
                               kind="ExternalInput")
    neg_out_d = nc.dram_tensor("neg_out", [D, RT_PER_CORE], F32,
                               kind="ExternalOutput")
    pos_out_d = nc.dram_tensor("pos_out", [D, RT_PER_CORE], F32,
                               kind="ExternalOutput")
    gneg_out_d = nc.dram_tensor("gneg_out", [TRN // 4, 4 * ROWS_PER_CORE], F32,
                                kind="ExternalOutput")

    with TileContext(nc) as tc:
        with (
            tc.tile_pool(name="big", bufs=1) as big,
            tc.tile_pool(name="rps", bufs=2, space="PSUM") as rps_pool,
            tc.tile_pool(name="tps", bufs=2, space="PSUM") as tps_pool,
            tc.tile_pool(name="scr", bufs=4) as scr,
            tc.tile_pool(name="tb", bufs=2) as tb_pool,
            tc.tile_pool(name="small", bufs=1) as small,
        ):
            featsT = big.tile([D, N], F32R, tag="featsT")
            sqb = big.tile([D, 4 * RW], F32R, tag="sqb")
            ones = small.tile([D, 128], F32R, tag="ones")
            rows2 = big.tile([D, ROWS_PER_CORE], F32R, tag="rows2")
            negsq = small.tile([D, TRN], F32, tag="negsq")
            negmask = big.tile([D, RT_PER_CORE * W], F32, tag="negmask")
            posmask = big.tile([D, RT_PER_CORE * W], F32, tag="posmask")
            # critical-path first; spread issue across the 3 DMA-capable queues
            nc.sync.dma_start(ones[:, :], ones_d[:, :])
            nc.gpsimd.dma_start(rows2[:, :], rows2_d[:, :])
            nc.scalar.dma_start(negsq[:, :], negsq_d[:, :])
            for ch in range(8):
                sl = slice(ch * 1024, (ch + 1) * 1024)
                eng = nc.sync if ch % 2 == 0 else nc.gpsimd
                eng.dma_start(featsT[:, sl], featsT_d[:, sl])
            for ch in range(4):
                sl = slice(ch * RW, (ch + 1) * RW)
                nc.scalar.dma_start(sqb[:, sl], sqb_d[:, sl])
            nc.sync.dma_start(negmask[:, :], negmask_d[:, :])
            nc.sync.dma_start(posmask[:, :], posmask_d[:, :])

            neg_sb = small.tile([D, RT_PER_CORE], F32, tag="neg_sb")
            pos_sb = small.tile([D, RT_PER_CORE], F32, tag="pos_sb")

            def emit_transposed(q):
                # 4 j-tiles -> one concatenated tbuf -> one partition_all_reduce
                tbuf = tb_pool.tile([D, 4 * ROWS_PER_CORE], F32, tag="tbuf",
                                    name=f"tbuf{q}")
                for u in range(4):
                    jt = 4 * q + u
                    lhsT = featsT[:, TR0 + 128 * jt:TR0 + 128 * (jt + 1)]
                    for h in range(2):
                        ps_t = tps_pool.tile([D, MMN], F32, tag="ps_t",
                                             name=f"ps_t{jt}_{h}")
                        nc.tensor.matmul(
                            ps_t[:, :], lhsT,
                            featsT[:, W // 2 + h * MMN:W // 2 + (h + 1) * MMN],
                            start=True, stop=True)
                        nc.scalar.activation(
                            tbuf[:, u * ROWS_PER_CORE + h * MMN:
                                 u * ROWS_PER_CORE + (h + 1) * MMN],
                            ps_t[:, :],
                            mybir.ActivationFunctionType.Identity,
                            bias=negsq[:, jt:jt + 1], scale=2.0)
                gout = tb_pool.tile([D, 4 * ROWS_PER_CORE], F32, tag="gout",
                                    name=f"gout{q}")
                nc.gpsimd.partition_all_reduce(
                    gout[:, :], tbuf[:, :], 128, bass_isa.ReduceOp.max)
                nc.sync.dma_start(gneg_out_d[q:q + 1, :], gout[0:1, :])

            for lt in range(RT_PER_CORE):
                if lt % 2 == 0:
                    emit_transposed(lt // 2)
                lhsT = rows2[:, 128 * lt:128 * (lt + 1)]
                partials = scr.tile([D, 4], F32, tag="partials",
                                    name=f"partials{lt}")
                w0 = 128 * lt + 64
                for g, (c0g, c1g) in enumerate(ROW_GROUPS):
                    ps = rps_pool.tile([D, RW], F32, tag="ps",
                                       name=f"ps{lt}_{g}")
                    for k in range(RW // MMN):
                        c0 = c0g + k * MMN
                        sq0 = g * RW + k * MMN
                        nc.tensor.matmul(
                            ps[:, k * MMN:(k + 1) * MMN], lhsT,
                            featsT[:, c0:c0 + MMN], start=True, stop=False)
                        nc.tensor.matmul(
                            ps[:, k * MMN:(k + 1) * MMN], ones,
                            sqb[:, sq0:sq0 + MMN], start=False, stop=True)
                    if g == 0:
                        scrP = scr.tile([D, W], F32, tag="scrP",
                                        name=f"scrP{lt}")
                        nc.vector.tensor_tensor(
                            out=scrP[:, :], in0=ps[:, w0:w0 + W],
                            in1=posmask[:, lt * W:(lt + 1) * W],
                            op=mybir.AluOpType.add)
                        nc.vector.tensor_reduce(
                            pos_sb[:, lt:lt + 1], scrP[:, :],
                            axis=mybir.AxisListType.X, op=mybir.AluOpType.max)
                        nc.vector.tensor_tensor(
                            out=ps[:, w0:w0 + W], in0=ps[:, w0:w0 + W],
                            in1=negmask[:, lt * W:(lt + 1) * W],
                            op=mybir.AluOpType.add)
                    nc.vector.tensor_reduce(
                        partials[:, g:g + 1], ps[:, :],
                        axis=mybir.AxisListType.X, op=mybir.AluOpType.min)
                nc.vector.tensor_reduce(
                    neg_sb[:, lt:lt + 1], partials[:, 0:4],
                    axis=mybir.AxisListType.X, op=mybir.AluOpType.min)

            nc.sync.dma_start(neg_out_d[:, :], neg_sb[:, :])
            nc.sync.dma_start(pos_out_d[:, :], pos_sb[:, :])

    nc.compile()
    return nc


def kernel(feats, labels):
    from concourse.bass_utils import run_bass_kernel_spmd

    feats = np.asarray(feats, dtype=np.float32)
    labels_np = np.asarray(labels).astype(np.int64)

    order = np.argsort(labels_np, kind="stable")
    feats_s = feats[order]
    labels_s = labels_np[order]

    counts = np.bincount(labels_s, minlength=max(int(labels_s.max()) + 1, 1))
    mc = int(counts.max())
    if mc <= 65:
        W = 256
    elif mc <= 129:
        W = 384
    elif mc <= 193:
        W = 512
    else:
        raise ValueError(f"class of size {mc} exceeds supported band window")

    if W not in _PROGRAM_CACHE:
        _PROGRAM_CACHE[W] = _build_program(W)
    nc = _PROGRAM_CACHE[W]

    sq = np.einsum("nd,nd->n", feats_s.astype(np.float64),
                   feats_s.astype(np.float64)).astype(np.float32)
    ones_np = np.ones((D, 128), dtype=np.float32)

    in_maps = []
    for c in range(NCORES):
        rot = (ROWS_PER_CORE * c - W // 2) % N
        loc = (rot + np.arange(N)) % N          # local col -> global sorted row
        featsT_c = np.ascontiguousarray(feats_s[loc].T)
        rows2_c = np.ascontiguousarray(
            (-2.0 * feats_s[ROWS_PER_CORE * c:ROWS_PER_CORE * (c + 1)]).T)
        sq_loc = sq[loc]
        rp_cols = np.concatenate([np.arange(a, b) for a, b in ROW_GROUPS])
        sqb_c = np.ascontiguousarray(
            np.broadcast_to((sq_loc[rp_cols] / 128.0)[None, :], (D, 4 * RW)))
        negsq_c = np.ascontiguousarray(
            -sq_loc[TR0:TR0 + TRN * 128].reshape(TRN, 128).T)
        negmask_c = np.zeros((D, RT_PER_CORE * W), dtype=np.float32)
        posmask_c = np.zeros((D, RT_PER_CORE * W), dtype=np.float32)
        for lt in range(RT_PER_CORE):
            rows_lab = labels_s[ROWS_PER_CORE * c + 128 * lt:
                                ROWS_PER_CORE * c + 128 * (lt + 1)]
            w0 = 128 * lt + 64
            win_lab = labels_s[loc[w0:w0 + W]]
            same = rows_lab[:, None] == win_lab[None, :]
            negmask_c[:, lt * W:(lt + 1) * W] = np.where(same, BIG, 0.0)
            posmask_c[:, lt * W:(lt + 1) * W] = np.where(same, 0.0, -BIG)
        in_maps.append({
            "featsT": featsT_c,
            "sqb": sqb_c,
            "ones": ones_np,
            "rows2": rows2_c,
            "negsq": negsq_c,
            "negmask": negmask_c,
            "posmask": posmask_c,
        })

    res = run_bass_kernel_spmd(nc, in_maps, core_ids=list(range(NCORES)))

    neg_raw = np.empty(N, dtype=np.float32)
    pos_raw = np.empty(N, dtype=np.float32)
    for c in range(NCORES):
        base = ROWS_PER_CORE * c
        nr = res.results[c]["neg_out"].T.reshape(ROWS_PER_CORE)
        tr = -res.results[c]["gneg_out"].reshape(TRN, ROWS_PER_CORE).max(axis=0)
        neg_raw[base:base + ROWS_PER_CORE] = np.minimum(nr, tr)
        pos_raw[base:base + ROWS_PER_CORE] = \
            res.results[c]["pos_out"].T.reshape(ROWS_PER_CORE)

    hn_sq = np.maximum(neg_raw + sq, 0.0).astype(np.float32)
    hp_sq = np.maximum(pos_raw + sq, 0.0).astype(np.float32)
    eps = np.float32(1e-12)
    hn = np.where(hn_sq > eps, np.sqrt(hn_sq), np.float32(0.0))
    hp = np.where(hp_sq > eps, np.sqrt(hp_sq), np.float32(0.0))

    cnt_per_row = counts[labels_s]
    valid = (cnt_per_row >= 2) & (cnt_per_row < N)
    diff = np.where(valid, hp - hn, np.float32(0.0))
    per_row = np.maximum(diff + np.float32(MARGIN), np.float32(0.0))
    per_row = np.where(valid, per_row, np.float32(0.0)).astype(np.float32)
    cnt = np.float32(valid.sum())
    if cnt > 0:
        loss = np.float32(per_row.sum(dtype=np.float32) / max(cnt, np.float32(1.0)))
    else:
        loss = np.float32(0.0)
    return np.float32(loss)


# revision 14
# speedup vs baseline: 1.2299x; 1.0580x over previous
"""BatchHardTripletLoss on 8 Trainium2 NeuronCores.

Strategy (data parallel over rows; all reductions in squared-distance space;
sqrt is monotone so squared-space hardest-pos/neg selection is exact):

  Host: sort rows by label. Core c owns sorted rows [1024c, 1024c+1024).
  Columns (all 8192 candidates) are rotated per core so its own rows sit at
  fixed local columns [W/2, W/2+1024) -> every row-tile's same-class columns
  fall in a fixed local window => one SPMD program for all 8 cores.

  Two device pipelines per core, split by column region:
   1) Row path (local cols [0,1536) u [3584,8192), includes the class band):
      TensorE: psum[i,j] = sq_j - 2 x_i.x_j  (f32r matmul with -2x rows as
      stationary + rank-1 ones @ sq/128 matmul accumulating sq_j);
      VectorE: min-reduce per 1536-col group; the band window gets +/-1e30
      label masks (tensor_tensor add) for hardest-neg / hardest-pos.
   2) Transposed path (local cols [1536,3584), guaranteed band-free):
      TensorE: psum[j,i] = x_j.x_i for 16 j-tiles x all 1024 own rows;
      ScalarE: tbuf = 2*psum - sq_j (per-partition bias, Identity activation);
      GpSimd:  partition_all_reduce(max) over the 128 j's -> per-jt row
      maxima, shipped to host which negates (min = -max(-t)) and combines.

  Host epilogue: + sq_i, clamp, sqrt (eps rule), validity from label counts
  (self-inclusion in hardest-pos is harmless: singleton classes are invalid
  by count), margin + masked mean in fp32.
"""

import numpy as np

N = 8192
D = 128
MARGIN = 0.3
NCORES = 8
ROWS_PER_CORE = N // NCORES          # 1024
RT_PER_CORE = ROWS_PER_CORE // 128   # 8 row-tiles
RW = 1536                            # row-path psum group width (3 banks)
TR0 = 1536                           # transposed region start (local cols)
TRN = 16                             # transposed j-tiles (128 each)
ROW_GROUPS = [(0, 1536), (3584, 5120), (5120, 6656), (6656, 8192)]
MMN = 512
BIG = 1.0e30

_PROGRAM_CACHE = {}


def _build_program(W):
    import concourse.mybir as mybir
    import concourse.bass_isa as bass_isa
    from concourse import bacc
    from concourse.tile import TileContext

    F32 = mybir.dt.float32
    F32R = mybir.dt.float32r

    nc = bacc.Bacc("TRN2", target_bir_lowering=False, debug=False,
                   num_devices=NCORES)

    featsT_d = nc.dram_tensor("featsT", [D, N], F32R, kind="ExternalInput")
    sqb_d = nc.dram_tensor("sqb", [D, 4 * RW], F32R, kind="ExternalInput")
    ones_d = nc.dram_tensor("ones", [D, 128], F32R, kind="ExternalInput")
    rows2_d = nc.dram_tensor("rows2", [D, ROWS_PER_CORE], F32R,
                             kind="ExternalInput")
    negsq_d = nc.dram_tensor("negsq", [D, TRN], F32, kind="ExternalInput")
    negmask_d = nc.dram_tensor("negmask", [D, RT_PER_CORE * W], F32,
                               kind="ExternalInput")
    posmask_d = nc.dram_tensor("posmask", [D, RT_PER_CORE * W], F32,
                               kind="ExternalInput")
    neg_out_d = nc.dram_tensor("neg_out", [D, RT_PER_CORE], F32,
                               kind="ExternalOutput")
    pos_out_d = nc.dram_tensor("pos_out", [D, RT_PER_CORE], F32,
                               kind="ExternalOutput")
    gneg_out_d = nc.dram_tensor("gneg_out", [TRN // 4, 4 * ROWS_PER_CORE], F32,
                                kind="ExternalOutput")

    with TileContext(nc) as tc:
        with (
            tc.tile_pool(name="big", bufs=1) as big,
            tc.tile_pool(name="rps", bufs=2, space="PSUM") as rps_pool,
            tc.tile_pool(name="tps", bufs=2, space="PSUM") as tps_pool,
            tc.tile_pool(name="scr", bufs=4) as scr,
            tc.tile_pool(name="tb", bufs=2) as tb_pool,
            tc.tile_pool(name="small", bufs=1) as small,
        ):
            featsT = big.tile([D, N], F32R, tag="featsT")
            sqb = big.tile([D, 4 * RW], F32R, tag="sqb")
            ones = small.tile([D, 128], F32R, tag="ones")
            rows2 = big.tile([D, ROWS_PER_CORE], F32R, tag="rows2")
            negsq = small.tile([D, TRN], F32, tag="negsq")
            negmask = big.tile([D, RT_PER_CORE * W], F32, tag="negmask")
            posmask = big.tile([D, RT_PER_CORE * W], F32, tag="posmask")
            # critical-path first; spread issue across the 3 DMA-capable queues
            nc.sync.dma_start(ones[:, :], ones_d[:, :])
            nc.gpsimd.dma_start(rows2[:, :], rows2_d[:, :])
            nc.scalar.dma_start(negsq[:, :], negsq_d[:, :])
            for ch in range(8):
                sl = slice(ch * 1024, (ch + 1) * 1024)
                eng = nc.sync if ch % 2 == 0 else nc.gpsimd
                eng.dma_start(featsT[:, sl], featsT_d[:, sl])
            for ch in range(4):
                sl = slice(ch * RW, (ch + 1) * RW)
                nc.scalar.dma_start(sqb[:, sl], sqb_d[:, sl])
            nc.sync.dma_start(negmask[:, :], negmask_d[:, :])
            nc.sync.dma_start(posmask[:, :], posmask_d[:, :])

            neg_sb = small.tile([D, RT_PER_CORE], F32, tag="neg_sb")
            pos_sb = small.tile([D, RT_PER_CORE], F32, tag="pos_sb")

            def emit_transposed(q):
                # 4 j-tiles -> one concatenated tbuf -> one partition_all_reduce
                tbuf = tb_pool.tile([D, 4 * ROWS_PER_CORE], F32, tag="tbuf",
                                    name=f"tbuf{q}")
                for u in range(4):
                    jt = 4 * q + u
                    lhsT = featsT[:, TR0 + 128 * jt:TR0 + 128 * (jt + 1)]
                    for h in range(2):
                        ps_t = tps_pool.tile([D, MMN], F32, tag="ps_t",
                                             name=f"ps_t{jt}_{h}")
                        nc.tensor.matmul(
                            ps_t[:, :], lhsT,
                            featsT[:, W // 2 + h * MMN:W // 2 + (h + 1) * MMN],
                            start=True, stop=True)
                        nc.scalar.activation(
                            tbuf[:, u * ROWS_PER_CORE + h * MMN:
                                 u * ROWS_PER_CORE + (h + 1) * MMN],
                            ps_t[:, :],
                            mybir.ActivationFunctionType.Identity,
                            bias=negsq[:, jt:jt + 1], scale=2.0)
                gout = tb_pool.tile([D, 4 * ROWS_PER_CORE], F32, tag="gout",
                                    name=f"gout{q}")
                nc.gpsimd.partition_all_reduce(
                    gout[:, :], tbuf[:, :], 128, bass_isa.ReduceOp.max)
                nc.sync.dma_start(gneg_out_d[q:q + 1, :], gout[0:1, :])

            for lt in range(RT_PER_CORE):
                if lt % 2 == 0:
                    emit_transposed(lt // 2)
                lhsT = rows2[:, 128 * lt:128 * (lt + 1)]
                partials = scr.tile([D, 4], F32, tag="partials",
                                    name=f"partials{lt}")
                w0 = 128 * lt + 64
                for g, (c0g, c1g) in enumerate(ROW_GROUPS):
                    ps = rps_pool.tile([D, RW], F32, tag="ps",
                                       name=f"ps{lt}_{g}")
                    for k in range(RW // MMN):
                        c0 = c0g + k * MMN
                        sq0 = g * RW + k * MMN
                        nc.tensor.matmul(
                            ps[:, k * MMN:(k + 1) * MMN], lhsT,
                            featsT[:, c0:c0 + MMN], start=True, stop=False)
                        nc.tensor.matmul(
                            ps[:, k * MMN:(k + 1) * MMN], ones,
                            sqb[:, sq0:sq0 + MMN], start=False, stop=True)
                    if g == 0:
                        scrP = scr.tile([D, W], F32, tag="scrP",
                                        name=f"scrP{lt}")
                        nc.vector.tensor_tensor(
                            out=scrP[:, :], in0=ps[:, w0:w0 + W],
                            in1=posmask[:, lt * W:(lt + 1) * W],
                            op=mybir.AluOpType.add)
                        nc.vector.tensor_reduce(
                            pos_sb[:, lt:lt + 1], scrP[:, :],
                            axis=mybir.AxisListType.X, op=mybir.AluOpType.max)
                        nc.vector.tensor_tensor(
                            out=ps[:, w0:w0 + W], in0=ps[:, w0:w0 + W],
                            in1=negmask[:, lt * W:(lt + 1) * W],
                            op=mybir.AluOpType.add)
                    nc.vector.tensor_reduce(
                        partials[:, g:g + 1], ps[:, :],
                        axis=mybir.AxisListType.X, op=mybir.AluOpType.min)
                nc.vector.tensor_reduce(
                    neg_sb[:, lt:lt + 1], partials[:, 0:4],
                    axis=mybir.AxisListType.X, op=mybir.AluOpType.min)

            nc.sync.dma_start(neg_out_d[:, :], neg_sb[:, :])
            nc.sync.dma_start(pos_out_d[:, :], pos_sb[:, :])

    nc.compile()
    return nc


def kernel(feats, labels):
    from concourse.bass_utils import run_bass_kernel_spmd

    feats = np.asarray(feats, dtype=np.float32)
    labels_np = np.asarray(labels).astype(np.int64)

    order = np.argsort(labels_np, kind="stable")
    feats_s = feats[order]
    labels_s = labels_np[order]

    counts = np.bincount(labels_s, minlength=max(int(labels_s.max()) + 1, 1))
    mc = int(counts.max())
    if mc <= 33:
        W = 192
    elif mc <= 65:
        W = 256
    elif mc <= 129:
        W = 384
    elif mc <= 193:
        W = 512
    else:
        raise ValueError(f"class of size {mc} exceeds supported band window")

    if W not in _PROGRAM_CACHE:
        _PROGRAM_CACHE[W] = _build_program(W)
    nc = _PROGRAM_CACHE[W]

    sq = np.einsum("nd,nd->n", feats_s.astype(np.float64),
                   feats_s.astype(np.float64)).astype(np.float32)
    ones_np = np.ones((D, 128), dtype=np.float32)

    in_maps = []
    for c in range(NCORES):
        rot = (ROWS_PER_CORE * c - W // 2) % N
        loc = (rot + np.arange(N)) % N          # local col -> global sorted row
        featsT_c = np.ascontiguousarray(feats_s[loc].T)
        rows2_c = np.ascontiguousarray(
            (-2.0 * feats_s[ROWS_PER_CORE * c:ROWS_PER_CORE * (c + 1)]).T)
        sq_loc = sq[loc]
        rp_cols = np.concatenate([np.arange(a, b) for a, b in ROW_GROUPS])
        sqb_c = np.ascontiguousarray(
            np.broadcast_to((sq_loc[rp_cols] / 128.0)[None, :], (D, 4 * RW)))
        negsq_c = np.ascontiguousarray(
            -sq_loc[TR0:TR0 + TRN * 128].reshape(TRN, 128).T)
        negmask_c = np.zeros((D, RT_PER_CORE * W), dtype=np.float32)
        posmask_c = np.zeros((D, RT_PER_CORE * W), dtype=np.float32)
        for lt in range(RT_PER_CORE):
            rows_lab = labels_s[ROWS_PER_CORE * c + 128 * lt:
                                ROWS_PER_CORE * c + 128 * (lt + 1)]
            w0 = 128 * lt + 64
            win_lab = labels_s[loc[w0:w0 + W]]
            same = rows_lab[:, None] == win_lab[None, :]
            negmask_c[:, lt * W:(lt + 1) * W] = np.where(same, BIG, 0.0)
            posmask_c[:, lt * W:(lt + 1) * W] = np.where(same, 0.0, -BIG)
        in_maps.append({
            "featsT": featsT_c,
            "sqb": sqb_c,
            "ones": ones_np,
            "rows2": rows2_c,
            "negsq": negsq_c,
            "negmask": negmask_c,
            "posmask": posmask_c,
        })

    res = run_bass_kernel_spmd(nc, in_maps, core_ids=list(range(NCORES)))

    neg_raw = np.empty(N, dtype=np.float32)
    pos_raw = np.empty(N, dtype=np.float32)
    for c in range(NCORES):
        base = ROWS_PER_CORE * c
        nr = res.results[c]["neg_out"].T.reshape(ROWS_PER_CORE)
        tr = -res.results[c]["gneg_out"].reshape(TRN, ROWS_PER_CORE).max(axis=0)
        neg_raw[base:base + ROWS_PER_CORE] = np.minimum(nr, tr)
        pos_raw[base:base + ROWS_PER_CORE] = \
            res.results[c]["pos_out"].T.reshape(ROWS_PER_CORE)

    hn_sq = np.maximum(neg_raw + sq, 0.0).astype(np.float32)
    hp_sq = np.maximum(pos_raw + sq, 0.0).astype(np.float32)
    eps = np.float32(1e-12)
    hn = np.where(hn_sq > eps, np.sqrt(hn_sq), np.float32(0.0))
    hp = np.where(hp_sq > eps, np.sqrt(hp_sq), np.float32(0.0))

    cnt_per_row = counts[labels_s]
    valid = (cnt_per_row >= 2) & (cnt_per_row < N)
    diff = np.where(valid, hp - hn, np.float32(0.0))
    per_row = np.maximum(diff + np.float32(MARGIN), np.float32(0.0))
    per_row = np.where(valid, per_row, np.float32(0.0)).astype(np.float32)
    cnt = np.float32(valid.sum())
    if cnt > 0:
        loss = np.float32(per_row.sum(dtype=np.float32) / max(cnt, np.float32(1.0)))
    else:
        loss = np.float32(0.0)
    return np.float32(loss)
